# revision 13
# baseline (speedup 1.0000x reference)
"""AuroraAttention Trainium2 kernel — 8-core SPMD, head-sharded, v3.

Strategy (tensor parallel over heads):
  - 16 heads -> 2 heads per core; both batches on every core.
  - Per core: q/k/v projections restricted to its 2 heads (column-parallel),
    full attention for its (batch, head) pairs, row-parallel output
    projection producing a partial [B, S, E] output; host sums the 8
    partials.
  - Scores are computed TRANSPOSED (S^T[k, q]) so the attention-weight
    matrix is laid out with the contraction dim (k) on partitions for the
    A@V matmul. A 64-wide ones block in the V operand makes the same
    matmul produce the softmax denominators broadcast across 64 partitions.
  - All main-path data stays bf16: any fp8 in q/k/v/pt translates ~1:1
    into relative output error (softmax output is itself a 1/sqrt(N)-scale
    weighted average, so per-weight quantization noise does NOT average
    away) — measured 6.5e-2 rms with an fp8 path vs the 2e-2 gate.

Differences vs the 273us baseline:
  - The exp affine folds the 1/sqrt(D) score scale and a -2 shift:
    pt = exp(s'/8 - 2); the shift cancels between numerator and
    denominator at normalize. Weights are NOT prescaled.
  - Bias handling is split per kt to balance engines: for kt in PE_KTS
    the PE adds a raw (x8) fp8 bias tile into the score PSUM via an
    identity matmul before exp (fp8 is safe for bias: its absolute
    magnitude is ~0.02 so quantization noise is ~7e-4 in nats); for the
    other kts the DVE multiplies exp(bias) (bf16) into pt after exp,
    exactly like the baseline.
  - Pass order is b-OUTER (batch-0 qb passes then batch-1), so batch-1's
    projections/transposes spread across passes 0-3 instead of cramming
    into one pass. Bias tiles are re-DMA'd per pass.
  - V^T -> V transposes go through the DMA XBAR (dma_start_transpose),
    not the PE: frees ~12.5us of PE time and 2 PSUM banks.
  - hidden[0] is DMA'd in four 512-column chunks so batch-0 projections
    (and pass 0) pipeline with the intake instead of waiting for the
    whole transfer.
  - Single oacc PSUM pool; A@V runs two kt behind the score stream so the
    previous pass's norm splices free the accumulator banks in time.
  - From pass 4 (projection pool released) the Wo matmuls get their own
    PSUM pool instead of stealing score-stream buffers.
"""

import numpy as np
import ml_dtypes

import concourse.bass as bass
import concourse.mybir as mybir
import concourse.tile as tile
from concourse.bass_utils import run_bass_kernel_spmd
from concourse.masks import make_identity
from bass_rust import SyncInfo

BF16 = ml_dtypes.bfloat16
FP8 = ml_dtypes.float8_e4m3fn
F32 = mybir.dt.float32
BF = mybir.dt.bfloat16
F8 = mybir.dt.float8e4
DR = mybir.MatmulPerfMode.DoubleRow

H, D, B, S, E = 16, 64, 2, 2048, 1024
N_CORES = 8
HPC = H // N_CORES  # heads per core
NQB = S // 512  # 4 q blocks
NKT = S // 128  # 16 k tiles
ECH = E // 128  # 8 contraction chunks for projections

# kts whose bias is added by the PE (fp8 identity matmul into PSUM); the
# rest multiply exp(bias) on the DVE after exp. Balances PE vs DVE load.
PE_KTS = (0, 1, 8)

# ---------------------------------------------------------------------------
# This walrus build rejects instructions carrying more than one sem wait
# ("Too many sync wait commands"). Tile freely emits multi-wait
# instructions, so after scheduling we move extra waits onto same-engine
# NoOps inserted immediately before the affected instruction. Engine
# streams execute in program order, so waiting on a preceding NoOp is
# semantically identical to waiting on the instruction itself.
_MAX_WAITS = 1


def split_multi_waits(nc: bass.Bass, max_waits: int = _MAX_WAITS):
    for bb in nc.main_func.blocks:
        lst = bb.instructions
        new = []
        changed = False
        for inst in lst:
            si = inst.sync_info
            if si is not None and si.on_wait and len(si.on_wait) > max_waits:
                waits = list(si.on_wait)
                extra, keep = waits[:-max_waits], waits[-max_waits:]
                for i in range(0, len(extra), max_waits):
                    nop = mybir.InstNoOp(
                        name=nc.get_next_instruction_name(), ins=[], outs=[]
                    )
                    nop.engine = inst.engine
                    nop.sync_info = SyncInfo(
                        on_wait=extra[i : i + max_waits], on_update=[]
                    )
                    nc.register_instruction(nop)
                    new.append(nop)
                inst.sync_info = SyncInfo(on_wait=keep, on_update=si.on_update)
                changed = True
            new.append(inst)
        if changed:
            bb.instructions = new
# ---------------------------------------------------------------------------


def build_nc() -> bass.Bass:
    nc = bass.Bass()

    # hidden^T packed partition-major [b, e', c, s]
    xt = nc.dram_tensor("xt", [B, 128, ECH, S], BF, kind="ExternalInput")
    # weights packed partition-major [e', c, dout], UNSCALED (the 1/8 score
    # scale lives in the exp affine)
    wq = nc.dram_tensor("wq", [128, ECH, 128], BF, kind="ExternalInput")
    wk = nc.dram_tensor("wk", [128, ECH, 128], BF, kind="ExternalInput")
    wv = nc.dram_tensor("wv", [128, ECH, 128], BF, kind="ExternalInput")
    bqkv = nc.dram_tensor("bqkv", [128, 3], F32, kind="ExternalInput")
    wo = nc.dram_tensor("wo", [128, E], BF, kind="ExternalInput")
    # PE-kt bias: raw bias x8, transposed: pb8[qb, kt, p, h*512+q'] =
    #     8 * bias[0, h, qb*512+q', kt*128+p]   (fp8)
    pb8 = nc.dram_tensor("pb8", [NQB, NKT, 128, 1024], F8, kind="ExternalInput")
    # DVE-kt bias: exp(bias), same transposed layout (bf16)
    peb = nc.dram_tensor("peb", [NQB, NKT, 128, 1024], BF, kind="ExternalInput")
    i8 = nc.dram_tensor("i8", [128, 128], F8, kind="ExternalInput")
    out = nc.dram_tensor("out", [B, S, E], BF, kind="ExternalOutput")

    with tile.TileContext(nc) as tc:
        _emit(tc, nc, xt, wq, wk, wv, bqkv, wo, pb8, peb, i8, out)
    split_multi_waits(nc)
    return nc


def _emit(tc, nc, xt, wq, wk, wv, bqkv, wo, pb8, peb, i8, out):
    with tc.tile_pool(name="persist", bufs=1) as persist:
        # ---- persistent SBUF tensors -----------------------------------
        w_sb = persist.tile([128, 3, ECH, 128], BF)  # WqT/WkT/WvT chunks
        b_sb = persist.tile([128, 3], F32)  # bq/bk/bv
        wo_sb = persist.tile([128, E], BF)  # Wo slice^T, both heads
        i8_sb = persist.tile([128, 128], F8)  # fp8 identity (bias adds)
        shift_c = persist.tile([128, 1], F32)  # exp shift constant (-2.0)
        qT_sb = persist.tile([128, B, S], BF)  # q^T (2 heads on partitions)
        kT_sb = persist.tile([128, B, S], BF)
        vT_sb = persist.tile([128, B, S], BF)  # v^T before transpose
        # v natural layout per k-tile: [v_h0 | ones64 | ones64 | v_h1]
        v_sb = persist.tile([128, B, NKT, 256], BF)
        o_norm = persist.tile([128, B, S], BF)  # normalized O^T, both heads
        ident = persist.tile([128, 128], BF)

        nc.vector.memset(shift_c, -2.0)
        nc.vector.memset(v_sb[:, :, :, 64:192], 1.0)
        make_identity(nc, ident)
        # preload the LN/EXP table set while startup DMAs stream
        warm = persist.tile([128, 16], BF)
        nc.scalar.activation(
            out=warm, in_=ident[:, 0:16], func=mybir.ActivationFunctionType.Exp
        )

        xt_sb = persist.tile([128, B, ECH, S], BF)

        # startup DMAs. hidden[0] goes in four 512-col chunks so batch-0
        # projections pipeline with the intake; chunk c unblocks the three
        # (proj, sblk=c) groups and the four V transposes of that s-range.
        for pi, w in enumerate((wq, wk, wv)):
            nc.sync.dma_start(out=w_sb[:, pi, :, :], in_=w[:, :, :])
        nc.sync.dma_start(out=xt_sb[:, 0, :, 0:512], in_=xt[0][:, :, 0:512])
        nc.scalar.dma_start(
            out=xt_sb[:, 0, :, 512:1024], in_=xt[0][:, :, 512:1024]
        )
        nc.sync.dma_start(out=i8_sb, in_=i8[:, :])
        nc.sync.dma_start(out=b_sb, in_=bqkv[:, :])
        nc.sync.dma_start(out=wo_sb, in_=wo[:, :])
        nc.scalar.dma_start(
            out=xt_sb[:, 0, :, 1024:1536], in_=xt[0][:, :, 1024:1536]
        )
        nc.sync.dma_start(
            out=xt_sb[:, 0, :, 1536:2048], in_=xt[0][:, :, 1536:2048]
        )
        nc.gpsimd.dma_start(out=xt_sb[:, 1, :, :], in_=xt[1])

        # ~3.4us of dummy matmuls while the first DMAs stream: HAM releases
        # the PE clock gate (1.2 -> 2.4 GHz) only after a sustained-busy
        # window, so warm up during time that is otherwise pure DMA wait
        with tc.tile_pool(name="warm_ps", bufs=1, space="PSUM") as warm_ps:
            wps = warm_ps.tile([128, 128], F32)
            for i in range(30):
                nc.tensor.matmul(
                    wps, lhsT=ident, rhs=ident, start=(i == 0), stop=(i == 29)
                )
            nc.vector.tensor_copy(out=warm, in_=wps[:, 0:16])

        dsts = (qT_sb, kT_sb, vT_sb)

        # ---- attention pools first (pool releases are stack-ordered; the
        # proj pool must sit on top of the PSUM stack to be releasable at
        # pass 4) ---------------------------------------------------------
        with (
            tc.tile_pool(name="eb_sb", bufs=4) as eb_pool,
            tc.tile_pool(name="b8_sb", bufs=2) as b8_pool,
            tc.tile_pool(name="pt_sb", bufs=8) as pt_pool,
            tc.tile_pool(name="norm_sb", bufs=1) as norm_pool,
            tc.tile_pool(name="wo_stage", bufs=4) as wo_stage,
            tc.tile_pool(name="sc_ps", bufs=2, space="PSUM") as sc_ps,
            tc.tile_pool(name="oacc_ps", bufs=1, space="PSUM") as oacc_ps,
        ):
            # proj pool (2 banks) lives through pass 3: batch-0 groups
            # emitted up front, batch-1 groups spliced into passes 0-3.
            proj_ps = tc.alloc_tile_pool(name="proj_ps", bufs=2, space="PSUM")
            # XBAR-transpose staging (SBUF, not PSUM)
            tp_pool = tc.alloc_tile_pool(name="tp_sb", bufs=2)

            def proj_group(pi, b, sblk):
                ps = proj_ps.tile([128, 512], F32, name="pj")
                for c in range(ECH):
                    nc.tensor.matmul(
                        ps,
                        lhsT=w_sb[:, pi, c, :],
                        rhs=xt_sb[:, b, c, sblk * 512 : (sblk + 1) * 512],
                        start=(c == 0),
                        stop=(c == ECH - 1),
                    )
                nc.vector.tensor_scalar_add(
                    dsts[pi][:, b, sblk * 512 : (sblk + 1) * 512],
                    ps,
                    b_sb[:, pi : pi + 1],
                )

            def v_transpose(b, st):
                # v^T -> v natural via the DMA XBAR (no PE, no PSUM). On
                # the scalar HWDGE queue: the sync queue carries the
                # per-pass bias stream and HWDGE is FIFO per queue — a
                # transpose waiting on its vT data must not head-of-line
                # block bias tiles.
                tp = tp_pool.tile([128, 128], BF, name="tp")
                nc.scalar.dma_start(
                    out=tp,
                    in_=vT_sb[:, b, st * 128 : (st + 1) * 128],
                    transpose=True,
                )
                nc.vector.tensor_copy(out=v_sb[:, b, st, 0:64], in_=tp[:, 0:64])
                nc.vector.tensor_copy(
                    out=v_sb[:, b, st, 192:256], in_=tp[:, 64:128]
                )

            # batch-0 startup work, in chunk-arrival order; k/q of the
            # first chunks first so pass 0 starts while later chunks stream
            for sblk in range(4):
                for pi in (1, 0, 2) if sblk == 0 else (1, 2):
                    proj_group(pi, 0, sblk)
                for st in range(4 * sblk, 4 * sblk + 4):
                    v_transpose(0, st)
            for sblk in (1, 2, 3):  # q blocks for passes 1-3
                proj_group(0, 0, sblk)

            # batch-1 work, spliced into passes 0-3
            b1_work: list = [
                (lambda sblk=sblk: proj_group(2, 1, sblk)) for sblk in range(4)
            ]
            b1_work.append(lambda: proj_group(1, 1, 0))
            for st in range(4):
                b1_work.append(lambda st=st: v_transpose(1, st))
            for sblk in (1, 2, 3):
                b1_work.append(lambda sblk=sblk: proj_group(1, 1, sblk))
                for st in range(4 * sblk, 4 * sblk + 4):
                    b1_work.append(lambda st=st: v_transpose(1, st))
            for sblk in range(4):
                b1_work.append(lambda sblk=sblk: proj_group(0, 1, sblk))

            wo_ps_holder: list = []  # dedicated Wo PSUM pool from pass 4

            def norm_den(oacc_b, rr):
                den = norm_pool.tile([128, 512], F32, name="den")
                nc.vector.tensor_copy(out=den[0:64, :], in_=oacc_b[0][64:128, :])
                nc.vector.tensor_copy(out=den[64:128, :], in_=oacc_b[1][0:64, :])
                rr["den"] = den

            def norm_ln(rr):
                ln = norm_pool.tile([128, 512], F32, name="ln")
                nc.scalar.activation(
                    out=ln, in_=rr["den"], func=mybir.ActivationFunctionType.Ln
                )
                rr["ln"] = ln

            def norm_rec(rr):
                r = norm_pool.tile([128, 512], F32, name="r")
                nc.scalar.activation(
                    out=r,
                    in_=rr["ln"],
                    func=mybir.ActivationFunctionType.Exp,
                    scale=-1.0,
                )
                rr["r"] = r

            def norm_chunk(qb, b, h, oacc_t, rr):
                qs = slice(qb * 512, (qb + 1) * 512)
                hp = slice(h * 64, (h + 1) * 64)
                nc.vector.tensor_mul(
                    out=o_norm[hp, b, qs], in0=oacc_t[hp, :], in1=rr["r"][hp, :]
                )

            wo_dma_q = [nc.gpsimd, nc.sync]

            def wo_chunk(qb, b, sti, tail=False):
                st = qb * 4 + sti
                stg = wo_stage.tile([128, E], BF, name="stg")
                if wo_ps_holder:
                    ps = wo_ps_holder[0].tile([128, E], F32, name="wop")
                else:
                    ps = sc_ps.tile([128, E], F32, name="sc")
                for eb in range(E // 512):
                    nc.tensor.matmul(
                        ps[:, eb * 512 : (eb + 1) * 512],
                        lhsT=o_norm[:, b, st * 128 : (st + 1) * 128],
                        rhs=wo_sb[:, eb * 512 : (eb + 1) * 512],
                        start=True,
                        stop=True,
                    )
                if not tail:
                    nc.vector.tensor_copy(out=stg, in_=ps)
                    wo_dma_q[sti % 2].dma_start(
                        out=out[b, st * 128 : (st + 1) * 128, :], in_=stg
                    )
                else:
                    # final drain: split copies across ACT+DVE and the DMAs
                    # across queues to cut the serial chain latency
                    nc.vector.tensor_copy(out=stg[:, 0:512], in_=ps[:, 0:512])
                    nc.scalar.copy(out=stg[:, 512:1024], in_=ps[:, 512:1024])
                    rows = slice(st * 128, (st + 1) * 128)
                    wo_dma_q[sti % 2].dma_start(
                        out=out[b, rows, 0:512], in_=stg[:, 0:512]
                    )
                    wo_dma_q[(sti + 1) % 2].dma_start(
                        out=out[b, rows, 512:1024], in_=stg[:, 512:1024]
                    )

            # per-kt splice budget: norm path first (frees oacc before this
            # pass's A@V start), Wo spread over the back half, batch-1
            # startup work (b1_work) drains through the _B1 slots
            _NORM_WO = {0: 1, 1: 2, 2: 2, 8: 1, 10: 1, 12: 1, 14: 1}
            _B1 = {3: 1, 4: 1, 5: 1, 6: 1, 7: 1, 9: 1, 11: 1, 13: 1, 15: 1}

            pending: list = []
            passes = [(b, qb) for b in range(B) for qb in range(NQB)]
            for pass_i, (b, qb) in enumerate(passes):
                qs = slice(qb * 512, (qb + 1) * 512)
                if pass_i == 4:
                    # all batch-1 projection groups drained (passes 0-3);
                    # repurpose the proj banks as a dedicated Wo pool so Wo
                    # chunks stop stealing score-stream PSUM buffers
                    assert not b1_work
                    proj_ps.release()
                    wo_ps_holder.append(
                        tc.alloc_tile_pool(name="wo_ps", bufs=1, space="PSUM")
                    )
                oacc_b = [
                    oacc_ps.tile([128, 512], F32, name=f"oacc_{h}")
                    for h in range(HPC)
                ]

                bias_tiles: dict = {}

                def load_bias(kt, qb=qb, bias_tiles=bias_tiles):
                    if kt in PE_KTS:
                        btile = b8_pool.tile([128, 1024], F8, name="b8")
                        nc.sync.dma_start(out=btile, in_=pb8[qb, kt])
                    else:
                        btile = eb_pool.tile([128, 1024], BF, name="eb")
                        nc.sync.dma_start(out=btile, in_=peb[qb, kt])
                    bias_tiles[kt] = btile

                for kt in range(3):
                    load_bias(kt)
                pt_tiles: dict = {}

                def av(kt, oacc_b=oacc_b, b=b, pt_tiles=pt_tiles):
                    pt = pt_tiles.pop(kt)
                    for h in range(HPC):
                        nc.tensor.matmul(
                            oacc_b[h],
                            lhsT=v_sb[:, b, kt, h * 128 : (h + 1) * 128],
                            rhs=pt[:, h * 512 : (h + 1) * 512],
                            start=(kt == 0),
                            stop=(kt == NKT - 1),
                        )

                for kt in range(NKT):
                    if kt + 3 < NKT:
                        load_bias(kt + 3)
                    ks = slice(kt * 128, (kt + 1) * 128)
                    pe_bias = kt in PE_KTS
                    s_ps = sc_ps.tile([128, 1024], F32, name="sc")
                    for h in range(HPC):
                        hp = slice(h * 64, (h + 1) * 64)
                        nc.tensor.matmul(
                            s_ps[:, h * 512 : (h + 1) * 512],
                            lhsT=kT_sb[hp, b, ks],
                            rhs=qT_sb[hp, b, qs],
                            start=True,
                            stop=not pe_bias,
                            skip_group_check=True,
                        )
                    if pe_bias:
                        # += 8*bias via fp8 identity matmul (absorbed by the
                        # 1/8 in the exp affine)
                        for h in range(HPC):
                            nc.tensor.matmul(
                                s_ps[:, h * 512 : (h + 1) * 512],
                                lhsT=i8_sb,
                                rhs=bias_tiles[kt][:, h * 512 : (h + 1) * 512],
                                start=False,
                                stop=True,
                                skip_group_check=True,
                            )
                    pt = pt_pool.tile([128, 1024], BF, name="pt")
                    # pt = exp(s'/8 [+ b] - 2)
                    nc.scalar.activation(
                        out=pt,
                        in_=s_ps,
                        func=mybir.ActivationFunctionType.Exp,
                        scale=0.125,
                        bias=shift_c[:, 0:1],
                    )
                    if not pe_bias:
                        nc.vector.tensor_mul(out=pt, in0=pt, in1=bias_tiles[kt])
                    pt_tiles[kt] = pt
                    # splices first, then A@V two kt behind: the previous
                    # pass's norm muls (slots 1-2) free the oacc banks just
                    # before av(0) is emitted
                    for _ in range(_NORM_WO.get(kt, 0)):
                        if pending:
                            pending.pop(0)()
                    for _ in range(_B1.get(kt, 0)):
                        if b1_work and (pass_i >= 1 or kt >= 10):
                            b1_work.pop(0)()
                    if kt >= 2:
                        av(kt - 2)
                av(NKT - 2)
                av(NKT - 1)
                while pending:
                    pending.pop(0)()
                rref: dict = {}
                last = pass_i == len(passes) - 1
                pending = (
                    [
                        lambda ob=oacc_b, rr=rref: norm_den(ob, rr),
                        lambda rr=rref: norm_ln(rr),
                        lambda rr=rref: norm_rec(rr),
                    ]
                    + [
                        (
                            lambda qb=qb, b=b, h=h, t=oacc_b[h], rr=rref: norm_chunk(
                                qb, b, h, t, rr
                            )
                        )
                        for h in range(HPC)
                    ]
                    + [
                        (
                            lambda qb=qb, b=b, sti=sti, tl=last: wo_chunk(
                                qb, b, sti, tail=tl
                            )
                        )
                        for sti in range(4)
                    ]
                )
            while pending:
                pending.pop(0)()
            if wo_ps_holder:
                wo_ps_holder[0].release()
            tp_pool.release()


# ---------------------------------------------------------------------------
# Host side


def make_in_maps(
    hidden_states, bias, Wq, bq, Wk, bk, Wv, bv, Wo
) -> list[dict[str, np.ndarray]]:
    hidden_states = np.asarray(hidden_states, np.float32)
    bias = np.asarray(bias, np.float32)

    # shared across cores
    xt = np.ascontiguousarray(
        hidden_states.transpose(0, 2, 1)  # [B, E, S]
        .reshape(B, ECH, 128, S)
        .transpose(0, 2, 1, 3)  # [B, 128, ECH, S] partition-major
    ).astype(BF16)
    i8 = np.eye(128, dtype=np.float32).astype(FP8)

    in_maps = []
    for c in range(N_CORES):
        rows = slice(c * HPC * D, (c + 1) * HPC * D)  # 128 output dims
        wq_c = np.asarray(Wq, np.float32)[rows, :].T  # [E, 128] unscaled
        wk_c = np.asarray(Wk, np.float32)[rows, :].T
        wv_c = np.asarray(Wv, np.float32)[rows, :].T
        bqkv_c = np.stack(
            [
                np.asarray(bq, np.float32)[rows],
                np.asarray(bk, np.float32)[rows],
                np.asarray(bv, np.float32)[rows],
            ],
            axis=1,
        )  # [128, 3]
        wo_c = np.asarray(Wo, np.float32)[:, rows].T  # [128, E]
        # transposed bias layouts [qb, kt, p(k), h*512+q']
        bc = bias[0, c * HPC : (c + 1) * HPC]  # [HPC, Sq, Sk]
        bt = bc.reshape(HPC, NQB, 512, NKT, 128).transpose(
            1, 3, 4, 0, 2
        )  # [qb, kt, p, h, q']
        pb8_c = np.ascontiguousarray(8.0 * bt.reshape(NQB, NKT, 128, 1024)).astype(
            FP8
        )
        peb_c = np.ascontiguousarray(
            np.exp(bt).reshape(NQB, NKT, 128, 1024)
        ).astype(BF16)

        in_maps.append(
            {
                "xt": xt,
                "wq": np.ascontiguousarray(
                    wq_c.reshape(ECH, 128, 128).transpose(1, 0, 2)
                ).astype(BF16),
                "wk": np.ascontiguousarray(
                    wk_c.reshape(ECH, 128, 128).transpose(1, 0, 2)
                ).astype(BF16),
                "wv": np.ascontiguousarray(
                    wv_c.reshape(ECH, 128, 128).transpose(1, 0, 2)
                ).astype(BF16),
                "bqkv": np.ascontiguousarray(bqkv_c),
                "wo": np.ascontiguousarray(wo_c).astype(BF16),
                "pb8": pb8_c,
                "peb": peb_c,
                "i8": i8,
            }
        )
    return in_maps


_NC_CACHE: list = []
LAST_RESULTS = None


def kernel(hidden_states, bias, Wq, bq, Wk, bk, Wv, bv, Wo) -> np.ndarray:
    global LAST_RESULTS
    if not _NC_CACHE:
        _NC_CACHE.append(build_nc())
    nc = _NC_CACHE[0]
    in_maps = make_in_maps(hidden_states, bias, Wq, bq, Wk, bk, Wv, bv, Wo)
    res = run_bass_kernel_spmd(nc, in_maps, list(range(N_CORES)))
    LAST_RESULTS = res
    total = np.zeros((B, S, E), np.float32)
    for c in range(N_CORES):
        total += np.asarray(res.results[c]["out"], np.float32)
    return total


# revision 22
# speedup vs baseline: 1.2222x; 1.2222x over previous
"""AuroraAttention Trainium2 kernel — 8-core SPMD, head-sharded, v3.

Strategy (tensor parallel over heads):
  - 16 heads -> 2 heads per core; both batches on every core.
  - Per core: q/k/v projections restricted to its 2 heads (column-parallel),
    full attention for its (batch, head) pairs, row-parallel output
    projection producing a partial [B, S, E] output; host sums the 8
    partials.
  - Scores are computed TRANSPOSED (S^T[k, q]) so the attention-weight
    matrix is laid out with the contraction dim (k) on partitions for the
    A@V matmul. A 64-wide ones block in the V operand makes the same
    matmul produce the softmax denominators broadcast across 64 partitions.
  - All main-path data stays bf16: any fp8 in q/k/v/pt translates ~1:1
    into relative output error (softmax output is itself a 1/sqrt(N)-scale
    weighted average, so per-weight quantization noise does NOT average
    away) — measured 6.5e-2 rms with an fp8 path vs the 2e-2 gate.

Differences vs the 273us baseline:
  - The exp affine folds the 1/sqrt(D) score scale and a -2 shift:
    pt = exp(s'/8 - 2); the shift cancels between numerator and
    denominator at normalize. Weights are NOT prescaled.
  - Bias handling is split per kt to balance engines: for kt in PE_KTS
    the PE adds a raw (x8) fp8 bias tile into the score PSUM via an
    identity matmul before exp (fp8 is safe for bias: its absolute
    magnitude is ~0.02 so quantization noise is ~7e-4 in nats); for the
    other kts the DVE multiplies exp(bias) (bf16) into pt after exp,
    exactly like the baseline.
  - Pass order is b-OUTER (batch-0 qb passes then batch-1), so batch-1's
    projections/transposes spread across passes 0-3 instead of cramming
    into one pass. Bias tiles are re-DMA'd per pass.
  - V^T -> V transposes go through the DMA XBAR (dma_start_transpose),
    not the PE: frees ~12.5us of PE time and 2 PSUM banks.
  - hidden[0] is DMA'd in four 512-column chunks so batch-0 projections
    (and pass 0) pipeline with the intake instead of waiting for the
    whole transfer.
  - Single oacc PSUM pool; A@V runs two kt behind the score stream so the
    previous pass's norm splices free the accumulator banks in time.
  - From pass 4 (projection pool released) the Wo matmuls get their own
    PSUM pool instead of stealing score-stream buffers.
"""

import numpy as np
import ml_dtypes

import concourse.bass as bass
import concourse.mybir as mybir
import concourse.tile as tile
from concourse.bass_utils import run_bass_kernel_spmd
from concourse.masks import make_identity
from bass_rust import SyncInfo

BF16 = ml_dtypes.bfloat16
FP8 = ml_dtypes.float8_e4m3fn
F32 = mybir.dt.float32
BF = mybir.dt.bfloat16
F8 = mybir.dt.float8e4
DR = mybir.MatmulPerfMode.DoubleRow

H, D, B, S, E = 16, 64, 2, 2048, 1024
N_CORES = 8
HPC = H // N_CORES  # heads per core
NQB = S // 512  # 4 q blocks
NKT = S // 128  # 16 k tiles
ECH = E // 128  # 8 contraction chunks for projections

# kts whose bias is added by the PE (fp8 identity matmul into PSUM); the
# rest multiply exp(bias) on the DVE after exp. Balances PE vs DVE load.
PE_KTS = (0, 1, 8)

# ---------------------------------------------------------------------------
# This walrus build rejects instructions carrying more than one sem wait
# ("Too many sync wait commands"). Tile freely emits multi-wait
# instructions, so after scheduling we move extra waits onto same-engine
# NoOps inserted immediately before the affected instruction. Engine
# streams execute in program order, so waiting on a preceding NoOp is
# semantically identical to waiting on the instruction itself.
_MAX_WAITS = 1


def split_multi_waits(nc: bass.Bass, max_waits: int = _MAX_WAITS):
    for bb in nc.main_func.blocks:
        lst = bb.instructions
        new = []
        changed = False
        for inst in lst:
            si = inst.sync_info
            if si is not None and si.on_wait and len(si.on_wait) > max_waits:
                waits = list(si.on_wait)
                extra, keep = waits[:-max_waits], waits[-max_waits:]
                for i in range(0, len(extra), max_waits):
                    nop = mybir.InstNoOp(
                        name=nc.get_next_instruction_name(), ins=[], outs=[]
                    )
                    nop.engine = inst.engine
                    nop.sync_info = SyncInfo(
                        on_wait=extra[i : i + max_waits], on_update=[]
                    )
                    nc.register_instruction(nop)
                    new.append(nop)
                inst.sync_info = SyncInfo(on_wait=keep, on_update=si.on_update)
                changed = True
            new.append(inst)
        if changed:
            bb.instructions = new
# ---------------------------------------------------------------------------


def build_nc() -> bass.Bass:
    nc = bass.Bass()

    # hidden^T packed partition-major and s-block-major [b, e', sblk, c, s']:
    # one 512-col chunk is 8KB contiguous per partition (128 fat DMA
    # descriptors, not 1024 thin ones)
    xt = nc.dram_tensor("xt", [B, 128, 4, ECH, 512], BF, kind="ExternalInput")
    # weights packed partition-major [e', c, dout], UNSCALED (the 1/8 score
    # scale lives in the exp affine)
    wq = nc.dram_tensor("wq", [128, ECH, 128], BF, kind="ExternalInput")
    wk = nc.dram_tensor("wk", [128, ECH, 128], BF, kind="ExternalInput")
    wv = nc.dram_tensor("wv", [128, ECH, 128], BF, kind="ExternalInput")
    bqkv = nc.dram_tensor("bqkv", [128, 3], F32, kind="ExternalInput")
    wo = nc.dram_tensor("wo", [128, E], BF, kind="ExternalInput")
    # PE-kt bias: raw bias x8, transposed: pb8[qb, kt, p, h*512+q'] =
    #     8 * bias[0, h, qb*512+q', kt*128+p]   (fp8)
    pb8 = nc.dram_tensor("pb8", [NQB, NKT, 128, 1024], F8, kind="ExternalInput")
    # DVE-kt bias: exp(bias), same transposed layout (bf16)
    peb = nc.dram_tensor("peb", [NQB, NKT, 128, 1024], BF, kind="ExternalInput")
    i8 = nc.dram_tensor("i8", [128, 128], F8, kind="ExternalInput")
    out = nc.dram_tensor("out", [B, S, E], BF, kind="ExternalOutput")

    with tile.TileContext(nc) as tc:
        _emit(tc, nc, xt, wq, wk, wv, bqkv, wo, pb8, peb, i8, out)
    split_multi_waits(nc)
    return nc


def _emit(tc, nc, xt, wq, wk, wv, bqkv, wo, pb8, peb, i8, out):
    with tc.tile_pool(name="persist", bufs=1) as persist:
        # ---- persistent SBUF tensors -----------------------------------
        w_sb = persist.tile([128, 3, ECH, 128], BF)  # WqT/WkT/WvT chunks
        b_sb = persist.tile([128, 3], F32)  # bq/bk/bv
        wo_sb = persist.tile([128, E], BF)  # Wo slice^T, both heads
        i8_sb = persist.tile([128, 128], F8)  # fp8 identity (bias adds)
        qT_sb = persist.tile([128, B, S], BF)  # q^T (2 heads on partitions)
        kT_sb = persist.tile([128, B, S], BF)
        vT_sb = persist.tile([128, B, S], BF)  # v^T before transpose
        # v natural layout per k-tile: [v_h0 | ones64 | ones64 | v_h1]
        v_sb = persist.tile([128, B, NKT, 256], BF)
        o_norm = persist.tile([128, B, S], BF)  # normalized O^T, both heads
        ident = persist.tile([128, 128], BF)

        nc.vector.memset(v_sb[:, :, :, 64:192], 1.0)
        make_identity(nc, ident)
        # preload the LN/EXP table set while startup DMAs stream
        warm = persist.tile([128, 16], BF)
        nc.scalar.activation(
            out=warm, in_=ident[:, 0:16], func=mybir.ActivationFunctionType.Exp
        )

        xt_sb = persist.tile([128, B, 4, ECH, 512], BF)

        # startup DMAs. hidden[0] goes in four 512-col chunks so batch-0
        # projections pipeline with the intake; chunk c unblocks the three
        # (proj, sblk=c) groups and the four V transposes of that s-range.
        for pi, w in enumerate((wq, wk, wv)):
            nc.sync.dma_start(out=w_sb[:, pi, :, :], in_=w[:, :, :])
        nc.sync.dma_start(out=xt_sb[:, 0, 0], in_=xt[0][:, 0])
        nc.scalar.dma_start(out=xt_sb[:, 0, 1], in_=xt[0][:, 1])
        nc.sync.dma_start(out=i8_sb, in_=i8[:, :])
        nc.sync.dma_start(out=b_sb, in_=bqkv[:, :])
        nc.sync.dma_start(out=wo_sb, in_=wo[:, :])
        nc.scalar.dma_start(out=xt_sb[:, 0, 2], in_=xt[0][:, 2])
        nc.sync.dma_start(out=xt_sb[:, 0, 3], in_=xt[0][:, 3])
        nc.gpsimd.dma_start(out=xt_sb[:, 1], in_=xt[1])

        # ~3.4us of dummy matmuls while the first DMAs stream: HAM releases
        # the PE clock gate (1.2 -> 2.4 GHz) only after a sustained-busy
        # window, so warm up during time that is otherwise pure DMA wait
        with tc.tile_pool(name="warm_ps", bufs=1, space="PSUM") as warm_ps:
            wps = warm_ps.tile([128, 128], F32)
            for i in range(30):
                nc.tensor.matmul(
                    wps, lhsT=ident, rhs=ident, start=(i == 0), stop=(i == 29)
                )
            nc.vector.tensor_copy(out=warm, in_=wps[:, 0:16])

        dsts = (qT_sb, kT_sb, vT_sb)

        # ---- attention pools first (pool releases are stack-ordered; the
        # proj pool must sit on top of the PSUM stack to be releasable at
        # pass 4) ---------------------------------------------------------
        with (
            tc.tile_pool(name="eb_sb", bufs=4) as eb_pool,
            tc.tile_pool(name="b8_sb", bufs=2) as b8_pool,
            tc.tile_pool(name="pt_sb", bufs=8) as pt_pool,
            tc.tile_pool(name="norm_sb", bufs=1) as norm_pool,
            tc.tile_pool(name="wo_stage", bufs=4) as wo_stage,
            tc.tile_pool(name="sc_ps", bufs=2, space="PSUM") as sc_ps,
            tc.tile_pool(name="oacc_ps", bufs=1, space="PSUM") as oacc_ps,
        ):
            # proj + transpose pools (1 bank each) live through pass 3:
            # groups/transposes are spliced one per kt slot, which already
            # serializes them, so single buffering costs nothing
            proj_ps = tc.alloc_tile_pool(name="proj_ps", bufs=1, space="PSUM")
            vtr_ps = tc.alloc_tile_pool(name="vtr_ps", bufs=1, space="PSUM")

            def proj_group(pi, b, sblk):
                ps = proj_ps.tile([128, 512], F32, name="pj")
                for c in range(ECH):
                    nc.tensor.matmul(
                        ps,
                        lhsT=w_sb[:, pi, c, :],
                        rhs=xt_sb[:, b, sblk, c, :],
                        start=(c == 0),
                        stop=(c == ECH - 1),
                    )
                nc.vector.tensor_scalar_add(
                    dsts[pi][:, b, sblk * 512 : (sblk + 1) * 512],
                    ps,
                    b_sb[:, pi : pi + 1],
                )

            def v_transpose(b, st):
                # v^T -> v natural (PE transpose per 128-wide s tile)
                tp = vtr_ps.tile([128, 128], BF, name="tp")
                nc.tensor.transpose(
                    out=tp,
                    in_=vT_sb[:, b, st * 128 : (st + 1) * 128],
                    identity=ident,
                )
                nc.vector.tensor_copy(out=v_sb[:, b, st, 0:64], in_=tp[:, 0:64])
                nc.vector.tensor_copy(
                    out=v_sb[:, b, st, 192:256], in_=tp[:, 64:128]
                )

            # minimal batch-0 work emitted ahead of pass 0 (engine queues
            # are FIFO — anything emitted here delays pass 0's first
            # scores, so only what kt0-3 need goes up front)
            proj_group(1, 0, 0)  # k sblk0
            proj_group(0, 0, 0)  # q qb0
            proj_group(2, 0, 0)  # v sblk0
            for st in range(4):
                v_transpose(0, st)
            proj_group(1, 0, 1)  # k sblk1

            # remaining batch-0 work: spliced into pass 0 (2 per kt slot)
            b0_work: list = []

            def _b0(fn, *a):
                b0_work.append(lambda: fn(*a))

            _b0(proj_group, 2, 0, 1)
            for st in (4, 5, 6, 7):
                _b0(v_transpose, 0, st)
            _b0(proj_group, 1, 0, 2)
            _b0(proj_group, 2, 0, 2)
            for st in (8, 9, 10, 11):
                _b0(v_transpose, 0, st)
            _b0(proj_group, 1, 0, 3)
            _b0(proj_group, 2, 0, 3)
            for st in (12, 13, 14, 15):
                _b0(v_transpose, 0, st)
            for sblk in (1, 2, 3):  # q blocks for passes 1-3
                _b0(proj_group, 0, 0, sblk)

            # batch-1 work, spliced into passes 0-3
            b1_work: list = [
                (lambda sblk=sblk: proj_group(2, 1, sblk)) for sblk in range(4)
            ]
            b1_work.append(lambda: proj_group(1, 1, 0))
            for st in range(4):
                b1_work.append(lambda st=st: v_transpose(1, st))
            for sblk in (1, 2, 3):
                b1_work.append(lambda sblk=sblk: proj_group(1, 1, sblk))
                for st in range(4 * sblk, 4 * sblk + 4):
                    b1_work.append(lambda st=st: v_transpose(1, st))
            for sblk in range(4):
                b1_work.append(lambda sblk=sblk: proj_group(0, 1, sblk))

            wo_ps_holder: list = []  # dedicated Wo PSUM pool from pass 4

            def norm_den(oacc_b, rr):
                den = norm_pool.tile([128, 512], F32, name="den")
                nc.vector.tensor_copy(out=den[0:64, :], in_=oacc_b[0][64:128, :])
                nc.vector.tensor_copy(out=den[64:128, :], in_=oacc_b[1][0:64, :])
                rr["den"] = den

            def norm_ln(rr):
                ln = norm_pool.tile([128, 512], F32, name="ln")
                nc.scalar.activation(
                    out=ln, in_=rr["den"], func=mybir.ActivationFunctionType.Ln
                )
                rr["ln"] = ln

            def norm_rec(rr):
                r = norm_pool.tile([128, 512], F32, name="r")
                nc.scalar.activation(
                    out=r,
                    in_=rr["ln"],
                    func=mybir.ActivationFunctionType.Exp,
                    scale=-1.0,
                )
                rr["r"] = r

            def norm_chunk(qb, b, h, oacc_t, rr):
                qs = slice(qb * 512, (qb + 1) * 512)
                hp = slice(h * 64, (h + 1) * 64)
                nc.vector.tensor_mul(
                    out=o_norm[hp, b, qs], in0=oacc_t[hp, :], in1=rr["r"][hp, :]
                )

            wo_dma_q = [nc.gpsimd, nc.sync]

            def wo_chunk(qb, b, sti, tail=False):
                st = qb * 4 + sti
                stg = wo_stage.tile([128, E], BF, name="stg")
                if wo_ps_holder:
                    ps = wo_ps_holder[0].tile([128, E], F32, name="wop")
                else:
                    ps = sc_ps.tile([128, E], F32, name="sc")
                for eb in range(E // 512):
                    nc.tensor.matmul(
                        ps[:, eb * 512 : (eb + 1) * 512],
                        lhsT=o_norm[:, b, st * 128 : (st + 1) * 128],
                        rhs=wo_sb[:, eb * 512 : (eb + 1) * 512],
                        start=True,
                        stop=True,
                    )
                if not tail:
                    nc.vector.tensor_copy(out=stg, in_=ps)
                    wo_dma_q[sti % 2].dma_start(
                        out=out[b, st * 128 : (st + 1) * 128, :], in_=stg
                    )
                else:
                    # final drain: split copies across ACT+DVE and the DMAs
                    # across queues to cut the serial chain latency
                    nc.vector.tensor_copy(out=stg[:, 0:512], in_=ps[:, 0:512])
                    nc.scalar.copy(out=stg[:, 512:1024], in_=ps[:, 512:1024])
                    rows = slice(st * 128, (st + 1) * 128)
                    wo_dma_q[sti % 2].dma_start(
                        out=out[b, rows, 0:512], in_=stg[:, 0:512]
                    )
                    wo_dma_q[(sti + 1) % 2].dma_start(
                        out=out[b, rows, 512:1024], in_=stg[:, 512:1024]
                    )

            # per-kt splice budget: norm path first (frees oacc before this
            # pass's A@V start), Wo spread over the back half, batch-1
            # startup work (b1_work) drains through the _B1 slots
            _NORM_WO = {0: 1, 1: 2, 2: 2, 8: 1, 10: 1, 12: 1, 14: 1}
            _B1 = {3: 1, 4: 1, 5: 1, 6: 1, 7: 1, 9: 1, 11: 1, 13: 1, 15: 1}

            pending: list = []
            passes = [(b, qb) for b in range(B) for qb in range(NQB)]
            for pass_i, (b, qb) in enumerate(passes):
                qs = slice(qb * 512, (qb + 1) * 512)
                if pass_i == 4:
                    # all batch-0/1 projection work drained (passes 0-3);
                    # repurpose the proj+vtr banks as a dedicated Wo pool so
                    # Wo chunks stop stealing score-stream PSUM buffers
                    assert not b0_work and not b1_work
                    vtr_ps.release()
                    proj_ps.release()
                    wo_ps_holder.append(
                        tc.alloc_tile_pool(name="wo_ps", bufs=1, space="PSUM")
                    )
                oacc_b = [
                    oacc_ps.tile([128, 512], F32, name=f"oacc_{h}")
                    for h in range(HPC)
                ]

                bias_tiles: dict = {}

                def load_bias(kt, qb=qb, bias_tiles=bias_tiles):
                    if kt in PE_KTS:
                        btile = b8_pool.tile([128, 1024], F8, name="b8")
                        nc.sync.dma_start(out=btile, in_=pb8[qb, kt])
                    else:
                        btile = eb_pool.tile([128, 1024], BF, name="eb")
                        nc.sync.dma_start(out=btile, in_=peb[qb, kt])
                    bias_tiles[kt] = btile

                for kt in range(3):
                    load_bias(kt)
                pt_tiles: dict = {}

                def av(kt, oacc_b=oacc_b, b=b, pt_tiles=pt_tiles):
                    pt = pt_tiles.pop(kt)
                    for h in range(HPC):
                        nc.tensor.matmul(
                            oacc_b[h],
                            lhsT=v_sb[:, b, kt, h * 128 : (h + 1) * 128],
                            rhs=pt[:, h * 512 : (h + 1) * 512],
                            start=(kt == 0),
                            stop=(kt == NKT - 1),
                        )

                for kt in range(NKT):
                    if kt + 3 < NKT:
                        load_bias(kt + 3)
                    ks = slice(kt * 128, (kt + 1) * 128)
                    pe_bias = kt in PE_KTS
                    s_ps = sc_ps.tile([128, 1024], F32, name="sc")
                    for h in range(HPC):
                        hp = slice(h * 64, (h + 1) * 64)
                        nc.tensor.matmul(
                            s_ps[:, h * 512 : (h + 1) * 512],
                            lhsT=kT_sb[hp, b, ks],
                            rhs=qT_sb[hp, b, qs],
                            start=True,
                            stop=not pe_bias,
                            skip_group_check=True,
                        )
                    if pe_bias:
                        # += 8*bias via fp8 identity matmul (absorbed by the
                        # 1/8 in the exp affine)
                        for h in range(HPC):
                            nc.tensor.matmul(
                                s_ps[:, h * 512 : (h + 1) * 512],
                                lhsT=i8_sb,
                                rhs=bias_tiles[kt][:, h * 512 : (h + 1) * 512],
                                start=False,
                                stop=True,
                                skip_group_check=True,
                            )
                    pt = pt_pool.tile([128, 1024], BF, name="pt")
                    # pt = exp(s'/8 [+ b]); no max-subtraction or shift:
                    # scores ~ N(0,1) + small bias, exp fits bf16 easily
                    nc.scalar.activation(
                        out=pt,
                        in_=s_ps,
                        func=mybir.ActivationFunctionType.Exp,
                        scale=0.125,
                    )
                    if not pe_bias:
                        nc.vector.tensor_mul(out=pt, in0=pt, in1=bias_tiles[kt])
                    pt_tiles[kt] = pt
                    # splices first, then A@V two kt behind: the previous
                    # pass's norm muls (slots 1-2) free the oacc banks just
                    # before av(0) is emitted
                    for _ in range(_NORM_WO.get(kt, 0)):
                        if pending:
                            pending.pop(0)()
                    if pass_i == 0:
                        # pass 0 absorbs the rest of batch-0's startup work
                        for _ in range(2):
                            if b0_work:
                                b0_work.pop(0)()
                            elif b1_work and kt >= 10:
                                b1_work.pop(0)()
                    else:
                        for _ in range(_B1.get(kt, 0)):
                            if b1_work:
                                b1_work.pop(0)()
                    if kt >= 2:
                        av(kt - 2)
                av(NKT - 2)
                av(NKT - 1)
                while pending:
                    pending.pop(0)()
                rref: dict = {}
                last = pass_i == len(passes) - 1
                pending = (
                    [
                        lambda ob=oacc_b, rr=rref: norm_den(ob, rr),
                        lambda rr=rref: norm_ln(rr),
                        lambda rr=rref: norm_rec(rr),
                    ]
                    + [
                        (
                            lambda qb=qb, b=b, h=h, t=oacc_b[h], rr=rref: norm_chunk(
                                qb, b, h, t, rr
                            )
                        )
                        for h in range(HPC)
                    ]
                    + [
                        (
                            lambda qb=qb, b=b, sti=sti, tl=last: wo_chunk(
                                qb, b, sti, tail=tl
                            )
                        )
                        for sti in range(4)
                    ]
                )
            while pending:
                pending.pop(0)()
            if wo_ps_holder:
                wo_ps_holder[0].release()


# ---------------------------------------------------------------------------
# Host side


def make_in_maps(
    hidden_states, bias, Wq, bq, Wk, bk, Wv, bv, Wo
) -> list[dict[str, np.ndarray]]:
    hidden_states = np.asarray(hidden_states, np.float32)
    bias = np.asarray(bias, np.float32)

    # shared across cores: [B, 128(p), 4(sblk), ECH(c), 512(s')]
    xt = np.ascontiguousarray(
        hidden_states.transpose(0, 2, 1)  # [B, E, S]
        .reshape(B, ECH, 128, 4, 512)
        .transpose(0, 2, 3, 1, 4)
    ).astype(BF16)
    i8 = np.eye(128, dtype=np.float32).astype(FP8)

    in_maps = []
    for c in range(N_CORES):
        rows = slice(c * HPC * D, (c + 1) * HPC * D)  # 128 output dims
        wq_c = np.asarray(Wq, np.float32)[rows, :].T  # [E, 128] unscaled
        wk_c = np.asarray(Wk, np.float32)[rows, :].T
        wv_c = np.asarray(Wv, np.float32)[rows, :].T
        bqkv_c = np.stack(
            [
                np.asarray(bq, np.float32)[rows],
                np.asarray(bk, np.float32)[rows],
                np.asarray(bv, np.float32)[rows],
            ],
            axis=1,
        )  # [128, 3]
        wo_c = np.asarray(Wo, np.float32)[:, rows].T  # [128, E]
        # transposed bias layouts [qb, kt, p(k), h*512+q']
        bc = bias[0, c * HPC : (c + 1) * HPC]  # [HPC, Sq, Sk]
        bt = bc.reshape(HPC, NQB, 512, NKT, 128).transpose(
            1, 3, 4, 0, 2
        )  # [qb, kt, p, h, q']
        pb8_c = np.ascontiguousarray(8.0 * bt.reshape(NQB, NKT, 128, 1024)).astype(
            FP8
        )
        peb_c = np.ascontiguousarray(
            np.exp(bt).reshape(NQB, NKT, 128, 1024)
        ).astype(BF16)

        in_maps.append(
            {
                "xt": xt,
                "wq": np.ascontiguousarray(
                    wq_c.reshape(ECH, 128, 128).transpose(1, 0, 2)
                ).astype(BF16),
                "wk": np.ascontiguousarray(
                    wk_c.reshape(ECH, 128, 128).transpose(1, 0, 2)
                ).astype(BF16),
                "wv": np.ascontiguousarray(
                    wv_c.reshape(ECH, 128, 128).transpose(1, 0, 2)
                ).astype(BF16),
                "bqkv": np.ascontiguousarray(bqkv_c),
                "wo": np.ascontiguousarray(wo_c).astype(BF16),
                "pb8": pb8_c,
                "peb": peb_c,
                "i8": i8,
            }
        )
    return in_maps


_NC_CACHE: list = []
LAST_RESULTS = None


def kernel(hidden_states, bias, Wq, bq, Wk, bk, Wv, bv, Wo) -> np.ndarray:
    global LAST_RESULTS
    if not _NC_CACHE:
        _NC_CACHE.append(build_nc())
    nc = _NC_CACHE[0]
    in_maps = make_in_maps(hidden_states, bias, Wq, bq, Wk, bk, Wv, bv, Wo)
    res = run_bass_kernel_spmd(nc, in_maps, list(range(N_CORES)))
    LAST_RESULTS = res
    total = np.zeros((B, S, E), np.float32)
    for c in range(N_CORES):
        total += np.asarray(res.results[c]["out"], np.float32)
    return total
